# revision 23
# baseline (speedup 1.0000x reference)
"""AttnBlock3d (GroupNorm + single-head self-attention + proj + residual)
on 8 Trainium2 NeuronCores.

Sharding: 8 shards = (batch sample b in 0..3) x (query-half h in 0..1).
Every core runs the SAME program (SPMD): the host permutes each sample's
N=4096 spatial columns so that the core's 2048 query positions come
first. GroupNorm / K / V are permutation-invariant in the column order,
and attention output for a query column does not depend on the ordering
of key columns, so the math is unchanged.

Key algebra (all exact; lets every big GEMM start without waiting for
the GroupNorm statistics):
  xn = A*x + Bvec per channel, A = inv_std*gamma, Bvec = beta - mean*A.
  The gamma factor is folded into the weights on the HOST
  (W' = W diag(gamma)), so q = Wq@xn + bq = inv_std*(Wq'@x) + bq2.
  Softmax over k is invariant to anything constant along k, so only the
  [k]-indexed part of the score bias survives; it comes out of two
  extra output columns of the V^T GEMM (h1.x, h2.x) combined with the
  stats. inv^2*SCALE enters through the ACT Exp per-partition scale.
  The V-side affine (v = inv*v_raw + cvv) is folded THROUGH the proj:
  out = WpT@(ph * inv/denom) + [Wp@cvv + Wp@bv + bp + x], with the
  channel constant dcc = dc1 + (-mean*inv)*pc2 built from host vectors
  dc1 = Wp@(bv + Wv@beta) + bp and pc2 = Wp@Wv@gamma, pre-added into
  the residual tiles.

Schedule (q-major two-phase sweep):
  - prologue: 2 packed weight DMAs + 8 x-piece DMAs; GroupNorm moments
    via PE column-sum matmuls + ACT Squares (both idle then); Q(H0)/K0
    GEMMs and fp8 copies so the first exp fires as soon as the stats
    chain resolves.
  - phase A: for kt in 0..31: scoresT(kt, query-half H0) -> 1024-wide
    Exp. Leftover K chunks / V^T tiles / Q(H1) GEMMs ride the PE+DVE
    slack early in A; AV chain for qc0 runs kt>=16 (PSUM frees then);
  - phase B: same over H1; AV for qc1 (burst) + qc2 (paced) and the
    divide/proj/residual tails for qc0..2 all inside the sweep.
  - epilogue: only qc3's AV + tail.
  PSUM: scores 2x[128,1024] (4 banks) + AV chain ph0,ph1,pd (3) +
  proj po (1) = 8, with prologue pools (stats, qkv) scoped to close
  before the chain/proj pools open.
"""

import numpy as np
from contextlib import ExitStack

import bass_rust
import concourse.bass as bass
import concourse.tile as tile
from concourse import mybir
from concourse.bass_utils import run_bass_kernel_spmd

F32 = mybir.dt.float32
F32R = mybir.dt.float32r
BF16 = mybir.dt.bfloat16
FP8 = mybir.dt.float8e4
AX = mybir.AxisListType
OP = mybir.AluOpType
AF = mybir.ActivationFunctionType

B, C, HH, WW, DD = 4, 256, 16, 16, 16
N = HH * WW * DD          # 4096 spatial positions per sample
NQ = N // 2               # 2048 query positions per core
P = 128                   # partitions
NCT = C // P              # 2 channel tiles
NKT = N // P              # 32 key tiles
QCH = 512                 # q-chunk width (one PSUM bank of fp32)
NQC = NQ // QCH           # 4 q chunks
EPS = 1e-6
SCALE = float(C) ** -0.5  # 0.0625
PACKW = 258 + 3 * C + 2   # wv_ext | wq | wk | wp | dc1 | pc2


def _split_excess_waits(nc, cap=1):
    """walrus in this env rejects >1 sync wait per instruction; peel
    extras onto no-ops inserted before the offender on the same engine."""
    n = 0
    for f in nc.m.functions:
        for blk in f.blocks:
            insts = blk.instructions
            new_insts = []
            for inst in insts:
                si = inst.sync_info
                if si is not None and si.on_wait is not None and len(si.on_wait) > cap:
                    waits = list(si.on_wait)
                    extra, keep = waits[:-cap], waits[-cap:]
                    for j in range(0, len(extra), cap):
                        nop = mybir.InstNoOp(
                            name=f"{inst.name}_ws{j}", ins=[], outs=[]
                        )
                        nop.engine = inst.engine
                        nop.sync_info = bass_rust.SyncInfo(
                            on_wait=extra[j : j + cap], on_update=[]
                        )
                        new_insts.append(nop)
                    inst.sync_info = bass_rust.SyncInfo(
                        on_wait=keep, on_update=list(si.on_update)
                    )
                    n += 1
                new_insts.append(inst)
            if len(new_insts) != len(insts):
                insts[:] = new_insts
    return n


def _r(ap):
    return ap.bitcast(F32R)


def _f(ap):
    return ap.bitcast(F32)


def build_program():
    nc = bass.Bass("TRN2", target_bir_lowering=False, debug=False)

    x_d = nc.dram_tensor("x", [C, N], F32R, kind="ExternalInput")
    w_d = nc.dram_tensor("wpack", [C, PACKW], F32R, kind="ExternalInput")
    out_d = nc.dram_tensor("out", [C, NQ], F32, kind="ExternalOutput")

    with tile.TileContext(nc) as tc, ExitStack() as ctx:
        # ---------- persistent pools ----------
        consts = ctx.enter_context(tc.tile_pool(name="consts", bufs=1))
        qk_pool = ctx.enter_context(tc.tile_pool(name="qk", bufs=1))
        vt_pool = ctx.enter_context(tc.tile_pool(name="vt", bufs=NKT // 2))
        xb_pool = ctx.enter_context(tc.tile_pool(name="xb", bufs=1))
        x_pool = ctx.enter_context(tc.tile_pool(name="xio", bufs=1))
        exp_pool = ctx.enter_context(tc.tile_pool(name="expt", bufs=NKT // 2))

        # packed weights: 2 DMAs total (one per channel tile)
        wpk = [consts.tile([P, PACKW], F32R, tag=f"wpk{ci}", name=f"wpk{ci}")
               for ci in range(NCT)]
        for ci in range(NCT):
            nc.sync.dma_start(wpk[ci][:], w_d.ap()[ci * P : (ci + 1) * P, :])
        wv_ext = [wpk[ci][:, 0:258] for ci in range(NCT)]
        wqA = [wpk[ci][:, 258 : 258 + C] for ci in range(NCT)]
        wkA = [wpk[ci][:, 258 + C : 258 + 2 * C] for ci in range(NCT)]
        wpT = [wpk[ci][:, 258 + 2 * C : 258 + 3 * C] for ci in range(NCT)]
        dc1 = [_f(wpk[ci][:, 258 + 3 * C : 259 + 3 * C]) for ci in range(NCT)]
        pc2 = [_f(wpk[ci][:, 259 + 3 * C : 260 + 3 * C]) for ci in range(NCT)]

        # x in 8 pieces of [128, 1024]: both channel tiles of the query
        # half first so Q/V^T GEMMs and stats start at ~1/4 of the load.
        xA = [x_pool.tile([P, NQ], F32R, tag=f"xA{ci}", name=f"xA{ci}")
              for ci in range(NCT)]
        xB = [x_pool.tile([P, NQ], F32R, tag=f"xB{ci}", name=f"xB{ci}")
              for ci in range(NCT)]
        x_pieces = []  # (ci, tile, col_lo, width) in DMA order; the
        # final piece is small so the last GroupNorm square (which gates
        # the stats chain) finishes right after the last DMA byte.
        plan = [(0, 0, 0, 1024), (0, 1, 0, 1024),
                (0, 0, 1024, 1024), (0, 1, 1024, 1024),
                (1, 0, 0, 1024), (1, 1, 0, 1024),
                (1, 0, 1024, 1024), (1, 1, 1024, 768), (1, 1, 1792, 256)]
        for half, ci, lo, w in plan:
            xt = (xA, xB)[half][ci]
            nc.sync.dma_start(
                xt[:, lo : lo + w],
                x_d.ap()[ci * P : (ci + 1) * P,
                         half * NQ + lo : half * NQ + lo + w])
            x_pieces.append((ci, xt, lo, w))

        def xs(ci, lo, w):
            """f32r view of x columns [lo, lo+w) (must not straddle NQ)."""
            if lo < NQ:
                assert lo + w <= NQ
                return xA[ci][:, lo : lo + w]
            return xB[ci][:, lo - NQ : lo - NQ + w]

        ones_colf = consts.tile([P, 1], F32, tag="ones_colf",
                                name="ones_colf")
        nc.vector.memset(ones_colf[:], 1.0)
        # f32r-rounded copy: fp32r matmul stationaries must come from a
        # rounded producer, and memset cannot write f32r directly
        ones_col = consts.tile([P, 1], F32R, tag="ones_col",
                               name="ones_col")
        nc.vector.tensor_copy(ones_col[:], ones_colf[:])
        ones_fp8 = consts.tile([P, 32], FP8, tag="ones_fp8", name="ones_fp8")
        nc.vector.memset(ones_fp8[:], 1.0)
        ones_row = consts.tile([1, P], F32, tag="ones_row", name="ones_row")
        nc.vector.memset(ones_row[:], 1.0)

        q2 = qk_pool.tile([P, NCT * NQ], FP8, tag="q2", name="q2")
        k2 = qk_pool.tile([P, NCT * N], FP8, tag="k2", name="k2")
        vT = [vt_pool.tile([P, 2 * C], FP8, tag="vt", name="vt")
              for _ in range(NKT // 2)]
        xbd = [xb_pool.tile([P, NQ], F32, tag=f"xb{ci}", name=f"xb{ci}")
               for ci in range(NCT)]
        hx = qk_pool.tile([P, 2 * NKT], F32, tag="hx", name="hx")
        hx3 = hx[:].rearrange("p (c k) -> p c k", c=2)
        bias_k = qk_pool.tile([P, NKT], F32, tag="bias_k", name="bias_k")
        # stats-derived broadcast columns:
        # [inv, -mean*inv, S*inv, S*inv^2, -S*mean*inv^2]
        mi_bc = consts.tile([P, 5], F32, tag="mi_bc", name="mi_bc")
        inv_row = consts.tile([1, P], F32, tag="inv_row", name="inv_row")
        dcc = [consts.tile([P, 1], F32, tag=f"dcc{ci}", name=f"dcc{ci}")
               for ci in range(NCT)]

        p_pre = ExitStack()
        st_pool = p_pre.enter_context(tc.tile_pool(name="stats", bufs=1))
        ps_st = p_pre.enter_context(
            tc.tile_pool(name="ps_st", bufs=1, space="PSUM"))
        ps_qkp = p_pre.enter_context(
            tc.tile_pool(name="ps_qkp", bufs=2, space="PSUM"))

        # ---------- GroupNorm moments ----------
        # column sums on the PE (idle in the prologue): 16 accumulating
        # [1,512] matmuls in x-piece DMA order; sums of squares on the
        # ACT (also idle) with the free-dim accumulator. Stats for piece
        # i are interleaved with the prologue GEMMs so at most two
        # not-yet-satisfied stat matmuls sit in the PE wait queue.
        colsum = ps_st.tile([1, QCH], F32, tag="colsum", name="colsum")
        NPC = len(x_pieces)
        q4 = st_pool.tile([P, NPC], F32, tag="q4", name="q4")
        scr = st_pool.tile([P, NQ // 2], F32, tag="scr", name="scr")

        def emit_stats(i):
            ci, xt, lo, w = x_pieces[i]
            with tc.high_priority():
                off = 0
                while off < w:
                    cw = min(QCH, w - off)
                    nc.tensor.matmul(
                        colsum[0:1, 0:cw], ones_col[:],
                        xt[:, lo + off : lo + off + cw],
                        start=(i == 0 and off == 0),
                        stop=(i == NPC - 1 and off + cw == w),
                        skip_group_check=True)
                    off += cw
                nc.scalar.activation(scr[:, 0:w], _f(xt[:, lo : lo + w]),
                                     AF.Square, accum_out=q4[:, i : i + 1])

        # ---------- prologue GEMMs (emission order = arrival order) ----
        def emit_vt(kt):
            lo = kt * P
            pv = ps_qkp.tile([P, C + 2], F32, tag="ps_v", name="ps_v")
            for ci in range(NCT):
                nc.tensor.matmul(pv[:], xs(ci, lo, P), wv_ext[ci],
                                 start=(ci == 0), stop=(ci == NCT - 1))
            # stash the two bias columns so pv can be released without
            # waiting for the stats; bias_k is batch-built later.
            nc.vector.tensor_copy(hx3[:, :, kt : kt + 1], pv[:, C : C + 2])
            nc.vector.tensor_copy(
                vT[kt // 2][:, (kt % 2) * C : (kt % 2 + 1) * C], pv[:, 0:C])

        def emit_qk(which, oc, col):
            w = wqA if which == "q" else wkA
            dst, width = (q2, NQ) if which == "q" else (k2, N)
            pk = ps_qkp.tile([P, QCH], F32, tag="ps_qk", name="ps_qk")
            for ci in range(NCT):
                nc.tensor.matmul(pk[:],
                                 w[ci][:, oc * P : (oc + 1) * P],
                                 xs(ci, col, QCH),
                                 start=(ci == 0), stop=(ci == NCT - 1))
            nc.vector.tensor_copy(
                dst[:, oc * width + col : oc * width + col + QCH], pk[:])

        # stats p0,p1 | V^T 0..3 + Q(qc0) | stats p2,p3 | V^T 4..7 +
        # Q(qc1) + K0 | stats p4..7 (all GEMMs need only x pieces 0,1)
        emit_stats(0)
        emit_stats(1)
        for kt in range(4):
            emit_vt(kt)
        for oc in range(NCT):
            emit_qk("q", oc, 0)
        emit_stats(2)
        emit_stats(3)
        for kt in range(4, 8):
            emit_vt(kt)
        for oc in range(NCT):
            emit_qk("q", oc, QCH)
        for oc in range(NCT):
            emit_qk("k", oc, 0)
        for i in range(4, NPC):
            emit_stats(i)

        # ---------- stats chain ----------
        # Entirely on the ACT engine (idle in the prologue, and immune
        # to the DVE copy congestion): activation computes
        # func(in*scale + bias) with per-partition AP scale, which gives
        # scalar-scalar multiply via scale=AP. The two tensor-tensor
        # combines that ACT cannot do (dcc, bias_k) go to GPSIMD.
        with tc.high_priority(offset=tc.cur_priority - 10):
            s_sum = st_pool.tile([1, 1], F32, tag="s_sum", name="s_sum")
            s_sq = st_pool.tile([1, 1], F32, tag="s_sq", name="s_sq")
            scr8 = st_pool.tile([1, NPC], F32, tag="scr8", name="scr8")
            scr512 = st_pool.tile([1, QCH], F32, tag="scr512", name="scr512")
            psq = ps_st.tile([1, NPC], F32, tag="psq", name="psq")
            nc.tensor.matmul(psq[:], ones_colf[:], q4[:])
            nc.scalar.activation(scr8[:], psq[:], AF.Copy,
                                 accum_out=s_sq[:])
            nc.scalar.activation(scr512[:], colsum[:], AF.Copy,
                                 accum_out=s_sum[:])

            inv_cn = 1.0 / float(C * N)
            mean_sb = st_pool.tile([1, 1], F32, tag="mean", name="mean")
            nc.scalar.activation(mean_sb[:], s_sum[:], AF.Copy,
                                 scale=inv_cn)
            msq = st_pool.tile([1, 1], F32, tag="msq", name="msq")
            nc.scalar.activation(msq[:], mean_sb[:], AF.Square)
            epsm = st_pool.tile([1, 1], F32, tag="epsm", name="epsm")
            nc.scalar.activation(epsm[:], msq[:], AF.Copy, scale=-1.0,
                                 bias=EPS)
            lnv = st_pool.tile([1, 1], F32, tag="lnv", name="lnv")
            nc.scalar.activation(lnv[:], s_sq[:], AF.Ln, scale=inv_cn,
                                 bias=epsm[:])
            mi_sb = st_pool.tile([1, 5], F32, tag="mi", name="mi")
            inv_c = mi_sb[:, 0:1]
            nc.scalar.activation(inv_c, lnv[:], AF.Exp, scale=-0.5)
            ninv = st_pool.tile([1, 1], F32, tag="ninv", name="ninv")
            nc.scalar.activation(ninv[:], inv_c, AF.Copy, scale=-1.0)
            nc.scalar.activation(mi_sb[:, 1:2], mean_sb[:], AF.Copy,
                                 scale=ninv[:])                  # -mean*inv
            nc.scalar.activation(mi_sb[:, 2:3], inv_c, AF.Copy,
                                 scale=SCALE)                    # S*inv
            nc.scalar.activation(mi_sb[:, 3:4], inv_c, AF.Copy,
                                 scale=mi_sb[:, 2:3])            # S*inv^2
            nc.scalar.activation(mi_sb[:, 4:5], mi_sb[:, 1:2], AF.Copy,
                                 scale=mi_sb[:, 2:3])            # -S*m*inv^2
            ps_bc5 = ps_st.tile([P, 5], F32, tag="ps_bc5", name="ps_bc5")
            nc.tensor.matmul(ps_bc5[:], ones_row[:], mi_sb[:])
            nc.scalar.activation(mi_bc[:], ps_bc5[:], AF.Copy)
            nc.vector.tensor_scalar(_r(inv_row[:]), ones_row[:],
                                    mi_sb[:, 0:1], None, op0=OP.mult)
            minv_neg = mi_bc[:, 1:2]
            si_bc = mi_bc[:, 2:3]
            si2_bc = mi_bc[:, 3:4]
            m2n_bc = mi_bc[:, 4:5]
            for ci in range(NCT):
                nc.gpsimd.tensor_scalar(dcc[ci][:], pc2[ci], minv_neg,
                                        dc1[ci], op0=OP.mult, op1=OP.add)
            # bias_k = S*inv*(h1.x) - S*mean*inv^2*(h2.x), batch 0..15
            # on the DVE (stt exists there; the first exps gate on it),
            # later batches on the idle GPSIMD (no stt -> 3 ops).
            bt = qk_pool.tile([P, 2 * NKT], F32, tag="bt", name="bt")

            def emit_bias(lo, hi):
                nc.gpsimd.tensor_scalar(bt[:, lo:hi], hx3[:, 0, lo:hi],
                                        si_bc, None, op0=OP.mult)
                nc.gpsimd.tensor_scalar(bt[:, 32 + lo : 32 + hi],
                                        hx3[:, 1, lo:hi], m2n_bc, None,
                                        op0=OP.mult)
                nc.gpsimd.tensor_tensor(bias_k[:, lo:hi], bt[:, lo:hi],
                                        bt[:, 32 + lo : 32 + hi], OP.add)

            nc.vector.tensor_scalar(bt[:, 0:16], hx3[:, 0, 0:16],
                                    si_bc, None, op0=OP.mult)
            nc.vector.scalar_tensor_tensor(bias_k[:, 0:16], hx3[:, 1, 0:16],
                                           m2n_bc, bt[:, 0:16],
                                           op0=OP.mult, op1=OP.add)

        p_pre.close()

        # ---------- sweep pools ----------
        p_sw = ExitStack()
        ps_s = p_sw.enter_context(
            tc.tile_pool(name="ps_s", bufs=2, space="PSUM"))
        p_qkv2 = ExitStack()
        ps_qk2 = p_qkv2.enter_context(
            tc.tile_pool(name="ps_qk2", bufs=2, space="PSUM"))

        def emit_vt2(kt):
            lo = kt * P
            pv = ps_qk2.tile([P, C + 2], F32, tag="ps_v2", name="ps_v2")
            for ci in range(NCT):
                nc.tensor.matmul(pv[:], xs(ci, lo, P), wv_ext[ci],
                                 start=(ci == 0), stop=(ci == NCT - 1))
            nc.vector.tensor_copy(hx3[:, :, kt : kt + 1], pv[:, C : C + 2])
            nc.vector.tensor_copy(
                vT[kt // 2][:, (kt % 2) * C : (kt % 2 + 1) * C], pv[:, 0:C])

        def emit_qk2(which, oc, col):
            w = wqA if which == "q" else wkA
            dst, width = (q2, NQ) if which == "q" else (k2, N)
            pk = ps_qk2.tile([P, QCH], F32, tag="ps_qk2t", name="ps_qk2t")
            for ci in range(NCT):
                nc.tensor.matmul(pk[:],
                                 w[ci][:, oc * P : (oc + 1) * P],
                                 xs(ci, col, QCH),
                                 start=(ci == 0), stop=(ci == NCT - 1))
            nc.vector.tensor_copy(
                dst[:, oc * width + col : oc * width + col + QCH], pk[:])

        k3all = k2[:].rearrange("p (j n) -> p j n", j=2)
        q3 = q2[:].rearrange("p (j n) -> p j n", j=2)

        def emit_scores_exp(kt, half):
            """scoresT + 1024-wide exp for (key tile kt, query half).
            High priority: the exp stream is the whole-kernel critical
            path, so its scores matmuls must win PE arbitration over AV
            bursts whenever both are ready."""
            if kt % 2 == 0 and half == 0:
                exp_tiles[kt // 2] = exp_pool.tile(
                    [P, 2 * NQ], FP8, tag="expt", name="expt")
            with tc.high_priority(offset=tc.cur_priority - 50):
                ps = ps_s.tile([P, 2 * QCH], F32, tag="s", name="s")
                k3 = k3all[:, :, kt * P : (kt + 1) * P]
                for qh in range(2):
                    qcol = half * 2 * QCH + qh * QCH
                    nc.tensor.matmul(
                        ps[:, qh * QCH : (qh + 1) * QCH],
                        k3, q3[:, :, qcol : qcol + QCH],
                        skip_group_check=True,
                        perf_mode=mybir.MatmulPerfMode.DoubleRow)
                lo = (kt % 2) * NQ + half * 2 * QCH
                nc.scalar.activation(
                    exp_tiles[kt // 2][:, lo : lo + 2 * QCH],
                    ps[:], AF.Exp, scale=si2_bc, bias=bias_k[:, kt : kt + 1])

        exp_tiles = [None] * (NKT // 2)
        ones3 = ones_fp8[:].rearrange("p (j o) -> p j o", j=2)[:, :, 0:1]

        # AV chain state (one chain at a time; 3 PSUM banks)
        p_ch = ExitStack()
        ch_h = None  # opened lazily at phase-A kt16

        def av_step(qc, p, ph, pd, first, last):
            et3 = exp_tiles[p].rearrange(
                "p (j q) -> p j q", j=2)[:, :, qc * QCH : (qc + 1) * QCH]
            vt3 = vT[p].rearrange("p (j c) -> p j c", j=2)
            for ct in range(NCT):
                nc.tensor.matmul(
                    ph[ct][:], vt3[:, :, ct * P : (ct + 1) * P], et3[:],
                    start=first, stop=last, skip_group_check=True,
                    perf_mode=mybir.MatmulPerfMode.DoubleRow)
            nc.tensor.matmul(
                pd[0:1, :], ones3, et3[:],
                start=first, stop=last, skip_group_check=True,
                perf_mode=mybir.MatmulPerfMode.DoubleRow)

        p_tail = ExitStack()

        with tc.tile_pool(name="att_sb", bufs=2) as att_pool, \
             tc.tile_pool(name="out_sb", bufs=4) as out_pool:

            tail_state = {}

            def tail_stage1(qc, ph, pd, cp_act=False):
                """recip + inv-scaled broadcast (DVE + PE). The
                broadcast lands back in the (now-free) denominator bank,
                so no extra PSUM bank is needed. In the epilogue the
                PSUM->SBUF copy goes to the idle ACT engine instead of
                the DVE, which is the epilogue's critical engine."""
                rec = att_pool.tile([1, QCH], F32, tag="rec", name="rec")
                with nc.allow_low_precision(reason="f32r fp32-width"):
                    nc.vector.reciprocal(_r(rec[:]), pd[0:1, :])
                nc.tensor.matmul(pd[:], _r(inv_row[:]), _r(rec[:]),
                                 skip_group_check=True)
                rec_bc = att_pool.tile([P, QCH], F32, tag="rec_bc",
                                       name="rec_bc")
                if cp_act:
                    nc.scalar.activation(rec_bc[:], pd[:], AF.Copy)
                else:
                    nc.vector.tensor_copy(rec_bc[:], pd[:])
                tail_state[qc] = (ph, rec_bc)

            def tail_stage2(qc):
                """h = ph * (inv/denom) into SBUF; releases the chain."""
                ph, rec_bc = tail_state[qc]
                h_sb = []
                for ct in range(NCT):
                    h = att_pool.tile([P, QCH], F32, tag=f"hsb{ct}",
                                      name=f"hsb{ct}")
                    nc.vector.tensor_tensor(_r(h[:]), ph[ct][:], rec_bc[:],
                                            OP.mult)
                    h_sb.append(h)
                tail_state[qc] = h_sb

            def tail_stage3(qc, oc, po=None):
                """proj GEMM + residual add + store for one oc. The
                epilogue passes explicit idle-bank APs for po so the
                proj GEMMs don't serialize on the single chain po bank."""
                h_sb = tail_state[qc]
                qsl = slice(qc * QCH, (qc + 1) * QCH)
                if po is None:
                    po = ch_h.tile([P, QCH], F32, tag="po", name="po")
                for ci in range(NCT):
                    nc.tensor.matmul(
                        po[:], wpT[ci][:, oc * P : (oc + 1) * P],
                        _r(h_sb[ci][:]),
                        start=(ci == 0), stop=(ci == NCT - 1),
                        skip_group_check=True)
                ot = out_pool.tile([P, QCH], F32, tag="ot", name="ot")
                nc.vector.tensor_tensor(ot[:], po[:], xbd[oc][:, qsl],
                                        OP.add)
                nc.sync.dma_start(out_d.ap()[oc * P : (oc + 1) * P, qsl],
                                  ot[:])

            # ================= phase A (query half 0) =================
            NP2 = NKT // 2
            ph_cur = pd_cur = None
            av_done = 0  # p index consumed for current chain
            for kt in range(NKT):
                # leftover GEMM injections: V^T 8..31 at kt 0..11 (2/kt),
                # K chunks 1..7 at kt 0..6; Q half-1 at kt 2..5; bias
                # batches (GPSIMD) once their hx columns have landed.
                if kt < 12:
                    emit_vt2(8 + 2 * kt)
                    emit_vt2(9 + 2 * kt)
                if kt < 7:
                    for oc in range(NCT):
                        emit_qk2("k", oc, (kt + 1) * QCH)
                if 2 <= kt < 6:
                    j = kt - 2
                    emit_qk2("q", j % 2, 2 * QCH + (j // 2) * QCH)
                if kt == 11:
                    emit_bias(16, 24)
                if kt == 14:
                    emit_bias(24, 32)
                if kt in (18, 20):
                    # residual tiles on the (idle) GPSIMD engine, emitted
                    # here so the scheduler keeps them off the bias path
                    ci = kt // 2 - 9
                    nc.gpsimd.tensor_scalar(xbd[ci][:], _f(xA[ci][:]),
                                            dcc[ci][:], None, op0=OP.add)
                if kt == 16:
                    p_qkv2.close()

                emit_scores_exp(kt, 0)

                if kt >= 17:
                    if kt == 17:
                        ch_h = p_ch.enter_context(
                            tc.tile_pool(name="ps_ch", bufs=1, space="PSUM"))
                        ph_cur = [ch_h.tile([P, QCH], F32, tag=f"h{ct}",
                                            name=f"h{ct}")
                                  for ct in range(NCT)]
                        pd_cur = ch_h.tile([P, QCH], F32, tag="d", name="d")
                    # consume p with exp done (2p+1 <= kt), max 2/step
                    target = min((kt - 1) // 2 + 1, NP2)
                    budget = 2
                    while av_done < target and budget > 0:
                        av_step(0, av_done, ph_cur, pd_cur,
                                av_done == 0, av_done == NP2 - 1)
                        av_done += 1
                        budget -= 1
            # finish qc0 chain (p15 needs the last A exp)
            while av_done < NP2:
                av_step(0, av_done, ph_cur, pd_cur,
                        av_done == 0, av_done == NP2 - 1)
                av_done += 1

            # ================= phase B (query half 1) =================
            tail_stage1(0, ph_cur, pd_cur)
            qc_av = 1        # chain currently running
            av_done = 0
            for kt in range(NKT):
                emit_scores_exp(kt, 1)
                if kt == 0:
                    tail_stage2(0)   # frees the qc0 chain PSUM
                if kt == 1:
                    tail_stage3(0, 0)
                if kt == 2:
                    tail_stage3(0, 1)
                # AV for qc1 (burst; all H0..no, all its exps exist) then
                # qc2 (paced behind the B exp sweep)
                if kt >= 1 and qc_av <= 2:
                    if qc_av == 1:
                        target = NP2
                        budget = 3
                    else:
                        target = min((kt - 1) // 2 + 1, NP2)
                        budget = 4
                    while av_done < target and budget > 0:
                        av_step(qc_av, av_done, ph_cur, pd_cur,
                                av_done == 0, av_done == NP2 - 1)
                        av_done += 1
                        budget -= 1
                    if av_done == NP2:
                        tail_stage1(qc_av, ph_cur, pd_cur,
                                    cp_act=(qc_av == 2))
                        tail_stage2(qc_av)
                        if qc_av == 1:
                            qc_av = 2
                            av_done = 0
                        else:
                            qc_av = 3
                if kt == 10:
                    tail_stage3(1, 0)
                if kt == 11:
                    tail_stage3(1, 1)
            # ================= epilogue: qc2 tail + qc3 ===============
            # qc3's denominator accumulates FIRST (the d bank frees as
            # soon as qc2's rec_bc is copied out), so its reciprocal +
            # broadcast chain overlaps the qc3 ph matmuls; epilogue proj
            # matmuls borrow idle scores banks to avoid po-bank churn.
            if qc_av == 2:
                while av_done < NP2:
                    av_step(2, av_done, ph_cur, pd_cur,
                            av_done == 0, av_done == NP2 - 1)
                    av_done += 1
                tail_stage1(2, ph_cur, pd_cur, cp_act=True)
                tail_stage2(2)

            # qc3's accumulators live in the now-idle scores banks so
            # its AV does not wait for qc2's tail to release the chain;
            # its denominator accumulates first so the reciprocal +
            # broadcast chain overlaps the ph matmuls.
            ph3 = ps_s.tile([P, 2 * QCH], F32, tag="s", name="ph3")
            po2 = ps_s.tile([P, 2 * QCH], F32, tag="s", name="po2")
            pd3 = ch_h.tile([P, QCH], F32, tag="po", name="pd3")

            def av3_pd(p, first, last):
                et3 = exp_tiles[p].rearrange(
                    "p (j q) -> p j q", j=2)[:, :, 3 * QCH : 4 * QCH]
                nc.tensor.matmul(
                    pd3[0:1, :], ones3, et3[:],
                    start=first, stop=last, skip_group_check=True,
                    perf_mode=mybir.MatmulPerfMode.DoubleRow)

            def av3_ph(p, first, last):
                et3 = exp_tiles[p].rearrange(
                    "p (j q) -> p j q", j=2)[:, :, 3 * QCH : 4 * QCH]
                vt3 = vT[p].rearrange("p (j c) -> p j c", j=2)
                for ct in range(NCT):
                    nc.tensor.matmul(
                        ph3[:, ct * QCH : (ct + 1) * QCH],
                        vt3[:, :, ct * P : (ct + 1) * P],
                        et3[:], start=first, stop=last,
                        skip_group_check=True,
                        perf_mode=mybir.MatmulPerfMode.DoubleRow)

            for p in range(NP2):
                av3_pd(p, p == 0, p == NP2 - 1)
            tail_stage1(3, None, pd3, cp_act=True)
            for p in range(NP2):
                av3_ph(p, p == 0, p == NP2 - 1)
            tail_stage3(2, 0, po=po2[:, 0:QCH])
            tail_stage3(2, 1, po=po2[:, QCH : 2 * QCH])
            tail_state[3] = ([ph3[:, 0:QCH], ph3[:, QCH : 2 * QCH]],
                             tail_state[3][1])
            tail_stage2(3)
            tail_stage3(3, 0, po=pd_cur[:])
            tail_stage3(3, 1, po=pd3[:])
            p_tail.close()
            p_ch.close()
        p_sw.close()

    _split_excess_waits(nc)
    return nc


def make_in_maps(x, norm_gamma, norm_beta, qkv_w, qkv_b, proj_w, proj_b):
    f = np.float32
    d = np.float64
    qkv_w = np.asarray(qkv_w, dtype=d)
    qkv_b = np.asarray(qkv_b, dtype=d)
    proj_w = np.asarray(proj_w, dtype=d)
    proj_b = np.asarray(proj_b, dtype=d)
    g = np.asarray(norm_gamma, dtype=d)
    beta = np.asarray(norm_beta, dtype=d)
    Wq, Wk, Wv = qkv_w[0:C], qkv_w[C : 2 * C], qkv_w[2 * C : 3 * C]
    bq, bk, bv = qkv_b[0:C], qkv_b[C : 2 * C], qkv_b[2 * C : 3 * C]

    wqT = (Wq.T * g[:, None])          # [c_in, c_out], rows scaled by gamma
    wkT = (Wk.T * g[:, None])
    wvT = (Wv.T * g[:, None])
    u1 = bq + Wq @ beta
    u2 = Wq @ g
    h1 = wkT @ u1
    h2 = wkT @ u2
    dc1 = proj_w @ (bv + Wv @ beta) + proj_b
    pc2 = proj_w @ (Wv @ g)

    wpack = np.zeros((C, PACKW), dtype=f)
    wpack[:, 0:C] = wvT
    wpack[:, C] = h1
    wpack[:, C + 1] = h2
    wpack[:, 258 : 258 + C] = wqT
    wpack[:, 258 + C : 258 + 2 * C] = wkT
    wpack[:, 258 + 2 * C : 258 + 3 * C] = proj_w.T
    wpack[:, 258 + 3 * C] = dc1
    wpack[:, 259 + 3 * C] = pc2
    wpack = np.ascontiguousarray(wpack)

    in_maps = []
    xf = np.asarray(x, dtype=f).reshape(B, C, N)
    for core in range(8):
        b, h = divmod(core, 2)
        xs = xf[b]
        if h == 1:
            xs = np.concatenate([xs[:, NQ:], xs[:, :NQ]], axis=1)
        in_maps.append({"x": np.ascontiguousarray(xs), "wpack": wpack})
    return in_maps


def assemble_output(results):
    out = np.empty((B, C, N), dtype=np.float32)
    for core in range(8):
        b, h = divmod(core, 2)
        out[b][:, h * NQ : (h + 1) * NQ] = results[core]["out"]
    return out.reshape(B, C, HH, WW, DD)


_PROGRAM = None
_N_CALLS = 0
_RUNNER = None


def get_program():
    global _PROGRAM
    if _PROGRAM is None:
        _PROGRAM = build_program()
    return _PROGRAM


def _build_cached_runner(nc):
    """Persistent jitted executor (same execution path that
    run_bass_kernel_spmd takes under axon, via bass2jax/PJRT) so repeat
    kernel() calls skip the multi-minute neuronx-cc recompile."""
    import jax
    from jax.experimental.shard_map import shard_map
    from jax.sharding import Mesh, PartitionSpec
    from concourse import bass2jax

    bass2jax.install_neuronx_cc_hook()
    n_cores = 8
    partition_name = (nc.partition_id_tensor.name
                      if nc.partition_id_tensor else None)
    in_names, out_names, out_avals, zero_outs = [], [], [], []
    for alloc in nc.m.functions[0].allocations:
        if not isinstance(alloc, mybir.MemoryLocationSet):
            continue
        name = alloc.memorylocations[0].name
        if alloc.kind == "ExternalInput":
            if name != partition_name:
                in_names.append(name)
        elif alloc.kind == "ExternalOutput":
            out_names.append(name)
            shape = tuple(alloc.tensor_shape)
            dtype = mybir.dt.np(alloc.dtype)
            out_avals.append(jax.core.ShapedArray(shape, dtype))
            zero_outs.append(np.zeros(shape, dtype))
    n_params = len(in_names)
    all_in_names = list(in_names) + list(out_names)
    if partition_name is not None:
        all_in_names.append(partition_name)

    def _body(*args):
        operands = list(args)
        if partition_name is not None:
            operands.append(bass2jax.partition_id_tensor())
        outs = bass2jax._bass_exec_p.bind(
            *operands,
            out_avals=tuple(out_avals),
            in_names=tuple(all_in_names),
            out_names=tuple(out_names),
            lowering_input_output_aliases=(),
            sim_require_finite=True,
            sim_require_nnan=True,
            nc=nc,
        )
        return tuple(outs)

    devices = jax.devices()[:n_cores]
    mesh = Mesh(np.asarray(devices), ("core",))
    n_outs = len(out_names)
    fn = jax.jit(
        shard_map(_body, mesh=mesh,
                  in_specs=(PartitionSpec("core"),) * (n_params + n_outs),
                  out_specs=(PartitionSpec("core"),) * n_outs,
                  check_rep=False),
        keep_unused=True,
    )

    def run(in_maps):
        per_core = [[np.asarray(m[name]) for name in in_names]
                    for m in in_maps]
        concat_in = [
            np.concatenate([per_core[c][i] for c in range(n_cores)], axis=0)
            for i in range(n_params)
        ]
        concat_zeros = [
            np.zeros((n_cores * z.shape[0], *z.shape[1:]), z.dtype)
            for z in zero_outs
        ]
        out_arrs = fn(*concat_in, *concat_zeros)
        return [
            {name: np.asarray(out_arrs[i]).reshape(
                n_cores, *out_avals[i].shape)[c]
             for i, name in enumerate(out_names)}
            for c in range(n_cores)
        ]

    return run


def kernel(x, norm_gamma, norm_beta, qkv_w, qkv_b, proj_w, proj_b):
    global _N_CALLS, _RUNNER
    nc = get_program()
    in_maps = make_in_maps(x, norm_gamma, norm_beta, qkv_w, qkv_b,
                           proj_w, proj_b)
    _N_CALLS += 1
    if _N_CALLS == 1:
        res = run_bass_kernel_spmd(nc, in_maps, core_ids=list(range(8)))
        return assemble_output(res.results)
    if _RUNNER is None:
        _RUNNER = _build_cached_runner(nc)
    return assemble_output(_RUNNER(in_maps))


# revision 24
# speedup vs baseline: 1.0022x; 1.0022x over previous
"""AttnBlock3d (GroupNorm + single-head self-attention + proj + residual)
on 8 Trainium2 NeuronCores.

Sharding: 8 shards = (batch sample b in 0..3) x (query-half h in 0..1).
Every core runs the SAME program (SPMD): the host permutes each sample's
N=4096 spatial columns so that the core's 2048 query positions come
first. GroupNorm / K / V are permutation-invariant in the column order,
and attention output for a query column does not depend on the ordering
of key columns, so the math is unchanged.

Key algebra (all exact; lets every big GEMM start without waiting for
the GroupNorm statistics):
  xn = A*x + Bvec per channel, A = inv_std*gamma, Bvec = beta - mean*A.
  The gamma factor is folded into the weights on the HOST
  (W' = W diag(gamma)), so q = Wq@xn + bq = inv_std*(Wq'@x) + bq2.
  Softmax over k is invariant to anything constant along k, so only the
  [k]-indexed part of the score bias survives; it comes out of two
  extra output columns of the V^T GEMM (h1.x, h2.x) combined with the
  stats. inv^2*SCALE enters through the ACT Exp per-partition scale.
  The V-side affine (v = inv*v_raw + cvv) is folded THROUGH the proj:
  out = WpT@(ph * inv/denom) + [Wp@cvv + Wp@bv + bp + x], with the
  channel constant dcc = dc1 + (-mean*inv)*pc2 built from host vectors
  dc1 = Wp@(bv + Wv@beta) + bp and pc2 = Wp@Wv@gamma, pre-added into
  the residual tiles.

Schedule (q-major two-phase sweep):
  - prologue: 2 packed weight DMAs + 8 x-piece DMAs; GroupNorm moments
    via PE column-sum matmuls + ACT Squares (both idle then); Q(H0)/K0
    GEMMs and fp8 copies so the first exp fires as soon as the stats
    chain resolves.
  - phase A: for kt in 0..31: scoresT(kt, query-half H0) -> 1024-wide
    Exp. Leftover K chunks / V^T tiles / Q(H1) GEMMs ride the PE+DVE
    slack early in A; AV chain for qc0 runs kt>=16 (PSUM frees then);
  - phase B: same over H1; AV for qc1 (burst) + qc2 (paced) and the
    divide/proj/residual tails for qc0..2 all inside the sweep.
  - epilogue: only qc3's AV + tail.
  PSUM: scores 2x[128,1024] (4 banks) + AV chain ph0,ph1,pd (3) +
  proj po (1) = 8, with prologue pools (stats, qkv) scoped to close
  before the chain/proj pools open.
"""

import numpy as np
from contextlib import ExitStack

import bass_rust
import concourse.bass as bass
import concourse.tile as tile
from concourse import mybir
from concourse.bass_utils import run_bass_kernel_spmd

F32 = mybir.dt.float32
F32R = mybir.dt.float32r
BF16 = mybir.dt.bfloat16
FP8 = mybir.dt.float8e4
AX = mybir.AxisListType
OP = mybir.AluOpType
AF = mybir.ActivationFunctionType

B, C, HH, WW, DD = 4, 256, 16, 16, 16
N = HH * WW * DD          # 4096 spatial positions per sample
NQ = N // 2               # 2048 query positions per core
P = 128                   # partitions
NCT = C // P              # 2 channel tiles
NKT = N // P              # 32 key tiles
QCH = 512                 # q-chunk width (one PSUM bank of fp32)
NQC = NQ // QCH           # 4 q chunks
EPS = 1e-6
SCALE = float(C) ** -0.5  # 0.0625
PACKW = 258 + 3 * C + 2   # wv_ext | wq | wk | wp | dc1 | pc2


def _split_excess_waits(nc, cap=1):
    """walrus in this env rejects >1 sync wait per instruction; peel
    extras onto no-ops inserted before the offender on the same engine."""
    n = 0
    for f in nc.m.functions:
        for blk in f.blocks:
            insts = blk.instructions
            new_insts = []
            for inst in insts:
                si = inst.sync_info
                if si is not None and si.on_wait is not None and len(si.on_wait) > cap:
                    waits = list(si.on_wait)
                    extra, keep = waits[:-cap], waits[-cap:]
                    for j in range(0, len(extra), cap):
                        nop = mybir.InstNoOp(
                            name=f"{inst.name}_ws{j}", ins=[], outs=[]
                        )
                        nop.engine = inst.engine
                        nop.sync_info = bass_rust.SyncInfo(
                            on_wait=extra[j : j + cap], on_update=[]
                        )
                        new_insts.append(nop)
                    inst.sync_info = bass_rust.SyncInfo(
                        on_wait=keep, on_update=list(si.on_update)
                    )
                    n += 1
                new_insts.append(inst)
            if len(new_insts) != len(insts):
                insts[:] = new_insts
    return n


def _r(ap):
    return ap.bitcast(F32R)


def _f(ap):
    return ap.bitcast(F32)


def build_program():
    nc = bass.Bass("TRN2", target_bir_lowering=False, debug=False)

    x_d = nc.dram_tensor("x", [C, N], F32R, kind="ExternalInput")
    w_d = nc.dram_tensor("wpack", [C, PACKW], F32R, kind="ExternalInput")
    out_d = nc.dram_tensor("out", [C, NQ], F32, kind="ExternalOutput")

    with tile.TileContext(nc) as tc, ExitStack() as ctx:
        # ---------- persistent pools ----------
        consts = ctx.enter_context(tc.tile_pool(name="consts", bufs=1))
        qk_pool = ctx.enter_context(tc.tile_pool(name="qk", bufs=1))
        vt_pool = ctx.enter_context(tc.tile_pool(name="vt", bufs=NKT // 2))
        xb_pool = ctx.enter_context(tc.tile_pool(name="xb", bufs=1))
        x_pool = ctx.enter_context(tc.tile_pool(name="xio", bufs=1))
        exp_pool = ctx.enter_context(tc.tile_pool(name="expt", bufs=NKT // 2))

        # packed weights: 2 DMAs total (one per channel tile)
        wpk = [consts.tile([P, PACKW], F32R, tag=f"wpk{ci}", name=f"wpk{ci}")
               for ci in range(NCT)]
        for ci in range(NCT):
            nc.sync.dma_start(wpk[ci][:], w_d.ap()[ci * P : (ci + 1) * P, :])
        wv_ext = [wpk[ci][:, 0:258] for ci in range(NCT)]
        wqA = [wpk[ci][:, 258 : 258 + C] for ci in range(NCT)]
        wkA = [wpk[ci][:, 258 + C : 258 + 2 * C] for ci in range(NCT)]
        wpT = [wpk[ci][:, 258 + 2 * C : 258 + 3 * C] for ci in range(NCT)]
        dc1 = [_f(wpk[ci][:, 258 + 3 * C : 259 + 3 * C]) for ci in range(NCT)]
        pc2 = [_f(wpk[ci][:, 259 + 3 * C : 260 + 3 * C]) for ci in range(NCT)]

        # x in 8 pieces of [128, 1024]: both channel tiles of the query
        # half first so Q/V^T GEMMs and stats start at ~1/4 of the load.
        xA = [x_pool.tile([P, NQ], F32R, tag=f"xA{ci}", name=f"xA{ci}")
              for ci in range(NCT)]
        xB = [x_pool.tile([P, NQ], F32R, tag=f"xB{ci}", name=f"xB{ci}")
              for ci in range(NCT)]
        x_pieces = []  # (ci, tile, col_lo, width) in DMA order; the
        # final piece is small so the last GroupNorm square (which gates
        # the stats chain) finishes right after the last DMA byte.
        plan = [(0, 0, 0, 1024), (0, 1, 0, 1024),
                (0, 0, 1024, 1024), (0, 1, 1024, 1024),
                (1, 0, 0, 1024), (1, 1, 0, 1024),
                (1, 0, 1024, 1024), (1, 1, 1024, 768), (1, 1, 1792, 256)]
        for half, ci, lo, w in plan:
            xt = (xA, xB)[half][ci]
            nc.sync.dma_start(
                xt[:, lo : lo + w],
                x_d.ap()[ci * P : (ci + 1) * P,
                         half * NQ + lo : half * NQ + lo + w])
            x_pieces.append((ci, xt, lo, w))

        def xs(ci, lo, w):
            """f32r view of x columns [lo, lo+w) (must not straddle NQ)."""
            if lo < NQ:
                assert lo + w <= NQ
                return xA[ci][:, lo : lo + w]
            return xB[ci][:, lo - NQ : lo - NQ + w]

        ones_colf = consts.tile([P, 1], F32, tag="ones_colf",
                                name="ones_colf")
        nc.vector.memset(ones_colf[:], 1.0)
        # f32r-rounded copy: fp32r matmul stationaries must come from a
        # rounded producer, and memset cannot write f32r directly
        ones_col = consts.tile([P, 1], F32R, tag="ones_col",
                               name="ones_col")
        nc.vector.tensor_copy(ones_col[:], ones_colf[:])
        ones_fp8 = consts.tile([P, 32], FP8, tag="ones_fp8", name="ones_fp8")
        nc.vector.memset(ones_fp8[:], 1.0)
        ones_row = consts.tile([1, P], F32, tag="ones_row", name="ones_row")
        nc.vector.memset(ones_row[:], 1.0)

        q2 = qk_pool.tile([P, NCT * NQ], FP8, tag="q2", name="q2")
        k2 = qk_pool.tile([P, NCT * N], FP8, tag="k2", name="k2")
        vT = [vt_pool.tile([P, 2 * C], FP8, tag="vt", name="vt")
              for _ in range(NKT // 2)]
        xbd = [xb_pool.tile([P, NQ], F32, tag=f"xb{ci}", name=f"xb{ci}")
               for ci in range(NCT)]
        hx = qk_pool.tile([P, 2 * NKT], F32, tag="hx", name="hx")
        hx3 = hx[:].rearrange("p (c k) -> p c k", c=2)
        bias_k = qk_pool.tile([P, NKT], F32, tag="bias_k", name="bias_k")
        # stats-derived broadcast columns:
        # [inv, -mean*inv, S*inv, S*inv^2, -S*mean*inv^2]
        mi_bc = consts.tile([P, 5], F32, tag="mi_bc", name="mi_bc")
        inv_row = consts.tile([1, P], F32, tag="inv_row", name="inv_row")
        dcc = [consts.tile([P, 1], F32, tag=f"dcc{ci}", name=f"dcc{ci}")
               for ci in range(NCT)]

        p_pre = ExitStack()
        st_pool = p_pre.enter_context(tc.tile_pool(name="stats", bufs=1))
        ps_st = p_pre.enter_context(
            tc.tile_pool(name="ps_st", bufs=1, space="PSUM"))
        ps_qkp = p_pre.enter_context(
            tc.tile_pool(name="ps_qkp", bufs=2, space="PSUM"))

        # ---------- GroupNorm moments ----------
        # column sums on the PE (idle in the prologue): 16 accumulating
        # [1,512] matmuls in x-piece DMA order; sums of squares on the
        # ACT (also idle) with the free-dim accumulator. Stats for piece
        # i are interleaved with the prologue GEMMs so at most two
        # not-yet-satisfied stat matmuls sit in the PE wait queue.
        colsum = ps_st.tile([1, QCH], F32, tag="colsum", name="colsum")
        NPC = len(x_pieces)
        q4 = st_pool.tile([P, NPC], F32, tag="q4", name="q4")
        scr = st_pool.tile([P, NQ // 2], F32, tag="scr", name="scr")

        sq_insts = []

        def emit_stats(i):
            ci, xt, lo, w = x_pieces[i]
            with tc.high_priority():
                off = 0
                while off < w:
                    cw = min(QCH, w - off)
                    nc.tensor.matmul(
                        colsum[0:1, 0:cw], ones_col[:],
                        xt[:, lo + off : lo + off + cw],
                        start=(i == 0 and off == 0),
                        stop=(i == NPC - 1 and off + cw == w),
                        skip_group_check=True)
                    off += cw
                sq_insts.append(
                    nc.scalar.activation(scr[:, 0:w], _f(xt[:, lo : lo + w]),
                                         AF.Square,
                                         accum_out=q4[:, i : i + 1]))

        # ---------- prologue GEMMs (emission order = arrival order) ----
        def emit_vt(kt):
            lo = kt * P
            pv = ps_qkp.tile([P, C + 2], F32, tag="ps_v", name="ps_v")
            for ci in range(NCT):
                nc.tensor.matmul(pv[:], xs(ci, lo, P), wv_ext[ci],
                                 start=(ci == 0), stop=(ci == NCT - 1))
            # stash the two bias columns so pv can be released without
            # waiting for the stats; bias_k is batch-built later.
            nc.vector.tensor_copy(hx3[:, :, kt : kt + 1], pv[:, C : C + 2])
            nc.vector.tensor_copy(
                vT[kt // 2][:, (kt % 2) * C : (kt % 2 + 1) * C], pv[:, 0:C])

        def emit_qk(which, oc, col):
            w = wqA if which == "q" else wkA
            dst, width = (q2, NQ) if which == "q" else (k2, N)
            pk = ps_qkp.tile([P, QCH], F32, tag="ps_qk", name="ps_qk")
            for ci in range(NCT):
                nc.tensor.matmul(pk[:],
                                 w[ci][:, oc * P : (oc + 1) * P],
                                 xs(ci, col, QCH),
                                 start=(ci == 0), stop=(ci == NCT - 1))
            nc.vector.tensor_copy(
                dst[:, oc * width + col : oc * width + col + QCH], pk[:])

        # stats p0,p1 | V^T 0..3 + Q(qc0) | stats p2,p3 | V^T 4..7 +
        # Q(qc1) + K0 | stats p4..7 (all GEMMs need only x pieces 0,1)
        emit_stats(0)
        emit_stats(1)
        for kt in range(4):
            emit_vt(kt)
        for oc in range(NCT):
            emit_qk("q", oc, 0)
        emit_stats(2)
        emit_stats(3)
        for kt in range(4, 8):
            emit_vt(kt)
        for oc in range(NCT):
            emit_qk("q", oc, QCH)
        for oc in range(NCT):
            emit_qk("k", oc, 0)
        for i in range(4, NPC):
            emit_stats(i)

        # ---------- stats chain ----------
        # Entirely on the ACT engine (idle in the prologue, and immune
        # to the DVE copy congestion): activation computes
        # func(in*scale + bias) with per-partition AP scale, which gives
        # scalar-scalar multiply via scale=AP. The two tensor-tensor
        # combines that ACT cannot do (dcc, bias_k) go to GPSIMD.
        with tc.high_priority(offset=tc.cur_priority - 10):
            s_sum = st_pool.tile([1, 1], F32, tag="s_sum", name="s_sum")
            s_sq = st_pool.tile([1, 1], F32, tag="s_sq", name="s_sq")
            scr512 = st_pool.tile([1, QCH], F32, tag="scr512", name="scr512")
            # sum of squares: cross-partition reduce of the 9 per-piece
            # accumulators on the (idle) GPSIMD engine
            nc.gpsimd.tensor_reduce(s_sq[:], q4[:], axis=AX.XYZWC,
                                    op=OP.add)
            # column-sum reduce on ACT; the nosync dep keeps the static
            # ACT order behind the last Square (the list scheduler's
            # internal timing model would otherwise flip them)
            sred = nc.scalar.activation(scr512[:], colsum[:], AF.Copy,
                                        accum_out=s_sum[:])
            dep = bass.InstructionNameOrderedSet()
            dep.add(sq_insts[-1].ins.name)
            sred.ins.add_nosync_dependencies_from(dep)

            inv_cn = 1.0 / float(C * N)
            mean_sb = st_pool.tile([1, 1], F32, tag="mean", name="mean")
            nc.scalar.activation(mean_sb[:], s_sum[:], AF.Copy,
                                 scale=inv_cn)
            msq = st_pool.tile([1, 1], F32, tag="msq", name="msq")
            nc.scalar.activation(msq[:], mean_sb[:], AF.Square)
            epsm = st_pool.tile([1, 1], F32, tag="epsm", name="epsm")
            nc.scalar.activation(epsm[:], msq[:], AF.Copy, scale=-1.0,
                                 bias=EPS)
            lnv = st_pool.tile([1, 1], F32, tag="lnv", name="lnv")
            nc.scalar.activation(lnv[:], s_sq[:], AF.Ln, scale=inv_cn,
                                 bias=epsm[:])
            mi_sb = st_pool.tile([1, 5], F32, tag="mi", name="mi")
            inv_c = mi_sb[:, 0:1]
            nc.scalar.activation(inv_c, lnv[:], AF.Exp, scale=-0.5)
            ninv = st_pool.tile([1, 1], F32, tag="ninv", name="ninv")
            nc.scalar.activation(ninv[:], inv_c, AF.Copy, scale=-1.0)
            nc.scalar.activation(mi_sb[:, 1:2], mean_sb[:], AF.Copy,
                                 scale=ninv[:])                  # -mean*inv
            nc.scalar.activation(mi_sb[:, 2:3], inv_c, AF.Copy,
                                 scale=SCALE)                    # S*inv
            nc.scalar.activation(mi_sb[:, 3:4], inv_c, AF.Copy,
                                 scale=mi_sb[:, 2:3])            # S*inv^2
            nc.scalar.activation(mi_sb[:, 4:5], mi_sb[:, 1:2], AF.Copy,
                                 scale=mi_sb[:, 2:3])            # -S*m*inv^2
            ps_bc5 = ps_st.tile([P, 5], F32, tag="ps_bc5", name="ps_bc5")
            nc.tensor.matmul(ps_bc5[:], ones_row[:], mi_sb[:])
            nc.scalar.activation(mi_bc[:], ps_bc5[:], AF.Copy)
            nc.vector.tensor_scalar(_r(inv_row[:]), ones_row[:],
                                    mi_sb[:, 0:1], None, op0=OP.mult)
            minv_neg = mi_bc[:, 1:2]
            si_bc = mi_bc[:, 2:3]
            si2_bc = mi_bc[:, 3:4]
            m2n_bc = mi_bc[:, 4:5]
            for ci in range(NCT):
                nc.gpsimd.tensor_scalar(dcc[ci][:], pc2[ci], minv_neg,
                                        dc1[ci], op0=OP.mult, op1=OP.add)
            # bias_k = S*inv*(h1.x) - S*mean*inv^2*(h2.x), batch 0..15
            # on the DVE (stt exists there; the first exps gate on it),
            # later batches on the idle GPSIMD (no stt -> 3 ops).
            bt = qk_pool.tile([P, 2 * NKT], F32, tag="bt", name="bt")

            def emit_bias(lo, hi):
                nc.gpsimd.tensor_scalar(bt[:, lo:hi], hx3[:, 0, lo:hi],
                                        si_bc, None, op0=OP.mult)
                nc.gpsimd.tensor_scalar(bt[:, 32 + lo : 32 + hi],
                                        hx3[:, 1, lo:hi], m2n_bc, None,
                                        op0=OP.mult)
                nc.gpsimd.tensor_tensor(bias_k[:, lo:hi], bt[:, lo:hi],
                                        bt[:, 32 + lo : 32 + hi], OP.add)

            nc.vector.tensor_scalar(bt[:, 0:16], hx3[:, 0, 0:16],
                                    si_bc, None, op0=OP.mult)
            nc.vector.scalar_tensor_tensor(bias_k[:, 0:16], hx3[:, 1, 0:16],
                                           m2n_bc, bt[:, 0:16],
                                           op0=OP.mult, op1=OP.add)

        p_pre.close()

        # ---------- sweep pools ----------
        p_sw = ExitStack()
        ps_s = p_sw.enter_context(
            tc.tile_pool(name="ps_s", bufs=2, space="PSUM"))
        p_qkv2 = ExitStack()
        ps_qk2 = p_qkv2.enter_context(
            tc.tile_pool(name="ps_qk2", bufs=2, space="PSUM"))

        def emit_vt2(kt):
            lo = kt * P
            pv = ps_qk2.tile([P, C + 2], F32, tag="ps_v2", name="ps_v2")
            for ci in range(NCT):
                nc.tensor.matmul(pv[:], xs(ci, lo, P), wv_ext[ci],
                                 start=(ci == 0), stop=(ci == NCT - 1))
            nc.vector.tensor_copy(hx3[:, :, kt : kt + 1], pv[:, C : C + 2])
            nc.vector.tensor_copy(
                vT[kt // 2][:, (kt % 2) * C : (kt % 2 + 1) * C], pv[:, 0:C])

        def emit_qk2(which, oc, col):
            w = wqA if which == "q" else wkA
            dst, width = (q2, NQ) if which == "q" else (k2, N)
            pk = ps_qk2.tile([P, QCH], F32, tag="ps_qk2t", name="ps_qk2t")
            for ci in range(NCT):
                nc.tensor.matmul(pk[:],
                                 w[ci][:, oc * P : (oc + 1) * P],
                                 xs(ci, col, QCH),
                                 start=(ci == 0), stop=(ci == NCT - 1))
            nc.vector.tensor_copy(
                dst[:, oc * width + col : oc * width + col + QCH], pk[:])

        k3all = k2[:].rearrange("p (j n) -> p j n", j=2)
        q3 = q2[:].rearrange("p (j n) -> p j n", j=2)

        def emit_scores_exp(kt, half):
            """scoresT + 1024-wide exp for (key tile kt, query half).
            High priority: the exp stream is the whole-kernel critical
            path, so its scores matmuls must win PE arbitration over AV
            bursts whenever both are ready."""
            if kt % 2 == 0 and half == 0:
                exp_tiles[kt // 2] = exp_pool.tile(
                    [P, 2 * NQ], FP8, tag="expt", name="expt")
            with tc.high_priority(offset=tc.cur_priority - 50):
                ps = ps_s.tile([P, 2 * QCH], F32, tag="s", name="s")
                k3 = k3all[:, :, kt * P : (kt + 1) * P]
                for qh in range(2):
                    qcol = half * 2 * QCH + qh * QCH
                    nc.tensor.matmul(
                        ps[:, qh * QCH : (qh + 1) * QCH],
                        k3, q3[:, :, qcol : qcol + QCH],
                        skip_group_check=True,
                        perf_mode=mybir.MatmulPerfMode.DoubleRow)
                lo = (kt % 2) * NQ + half * 2 * QCH
                nc.scalar.activation(
                    exp_tiles[kt // 2][:, lo : lo + 2 * QCH],
                    ps[:], AF.Exp, scale=si2_bc, bias=bias_k[:, kt : kt + 1])

        exp_tiles = [None] * (NKT // 2)
        ones3 = ones_fp8[:].rearrange("p (j o) -> p j o", j=2)[:, :, 0:1]

        # AV chain state (one chain at a time; 3 PSUM banks)
        p_ch = ExitStack()
        ch_h = None  # opened lazily at phase-A kt16

        def av_step(qc, p, ph, pd, first, last):
            et3 = exp_tiles[p].rearrange(
                "p (j q) -> p j q", j=2)[:, :, qc * QCH : (qc + 1) * QCH]
            vt3 = vT[p].rearrange("p (j c) -> p j c", j=2)
            for ct in range(NCT):
                nc.tensor.matmul(
                    ph[ct][:], vt3[:, :, ct * P : (ct + 1) * P], et3[:],
                    start=first, stop=last, skip_group_check=True,
                    perf_mode=mybir.MatmulPerfMode.DoubleRow)
            nc.tensor.matmul(
                pd[0:1, :], ones3, et3[:],
                start=first, stop=last, skip_group_check=True,
                perf_mode=mybir.MatmulPerfMode.DoubleRow)

        p_tail = ExitStack()

        with tc.tile_pool(name="att_sb", bufs=2) as att_pool, \
             tc.tile_pool(name="out_sb", bufs=4) as out_pool:

            tail_state = {}

            def tail_stage1(qc, ph, pd, cp_act=False):
                """recip + inv-scaled broadcast (DVE + PE). The
                broadcast lands back in the (now-free) denominator bank,
                so no extra PSUM bank is needed. In the epilogue the
                PSUM->SBUF copy goes to the idle ACT engine instead of
                the DVE, which is the epilogue's critical engine."""
                rec = att_pool.tile([1, QCH], F32, tag="rec", name="rec")
                with nc.allow_low_precision(reason="f32r fp32-width"):
                    nc.vector.reciprocal(_r(rec[:]), pd[0:1, :])
                nc.tensor.matmul(pd[:], _r(inv_row[:]), _r(rec[:]),
                                 skip_group_check=True)
                rec_bc = att_pool.tile([P, QCH], F32, tag="rec_bc",
                                       name="rec_bc")
                if cp_act:
                    nc.scalar.activation(rec_bc[:], pd[:], AF.Copy)
                else:
                    nc.vector.tensor_copy(rec_bc[:], pd[:])
                tail_state[qc] = (ph, rec_bc)

            def tail_stage2(qc):
                """h = ph * (inv/denom) into SBUF; releases the chain."""
                ph, rec_bc = tail_state[qc]
                h_sb = []
                for ct in range(NCT):
                    h = att_pool.tile([P, QCH], F32, tag=f"hsb{ct}",
                                      name=f"hsb{ct}")
                    nc.vector.tensor_tensor(_r(h[:]), ph[ct][:], rec_bc[:],
                                            OP.mult)
                    h_sb.append(h)
                tail_state[qc] = h_sb

            def tail_stage3(qc, oc, po=None):
                """proj GEMM + residual add + store for one oc. The
                epilogue passes explicit idle-bank APs for po so the
                proj GEMMs don't serialize on the single chain po bank."""
                h_sb = tail_state[qc]
                qsl = slice(qc * QCH, (qc + 1) * QCH)
                if po is None:
                    po = ch_h.tile([P, QCH], F32, tag="po", name="po")
                for ci in range(NCT):
                    nc.tensor.matmul(
                        po[:], wpT[ci][:, oc * P : (oc + 1) * P],
                        _r(h_sb[ci][:]),
                        start=(ci == 0), stop=(ci == NCT - 1),
                        skip_group_check=True)
                ot = out_pool.tile([P, QCH], F32, tag="ot", name="ot")
                nc.vector.tensor_tensor(ot[:], po[:], xbd[oc][:, qsl],
                                        OP.add)
                nc.sync.dma_start(out_d.ap()[oc * P : (oc + 1) * P, qsl],
                                  ot[:])

            # ================= phase A (query half 0) =================
            NP2 = NKT // 2
            ph_cur = pd_cur = None
            av_done = 0  # p index consumed for current chain
            for kt in range(NKT):
                # leftover GEMM injections: V^T 8..31 at kt 0..11 (2/kt),
                # K chunks 1..7 at kt 0..6; Q half-1 at kt 2..5; bias
                # batches (GPSIMD) once their hx columns have landed.
                if kt < 12:
                    emit_vt2(8 + 2 * kt)
                    emit_vt2(9 + 2 * kt)
                if kt < 7:
                    for oc in range(NCT):
                        emit_qk2("k", oc, (kt + 1) * QCH)
                if 2 <= kt < 6:
                    j = kt - 2
                    emit_qk2("q", j % 2, 2 * QCH + (j // 2) * QCH)
                if kt == 11:
                    emit_bias(16, 24)
                if kt == 14:
                    emit_bias(24, 32)
                if kt in (18, 20):
                    # residual tiles on the (idle) GPSIMD engine, emitted
                    # here so the scheduler keeps them off the bias path
                    ci = kt // 2 - 9
                    nc.gpsimd.tensor_scalar(xbd[ci][:], _f(xA[ci][:]),
                                            dcc[ci][:], None, op0=OP.add)
                if kt == 16:
                    p_qkv2.close()

                emit_scores_exp(kt, 0)

                if kt >= 17:
                    if kt == 17:
                        ch_h = p_ch.enter_context(
                            tc.tile_pool(name="ps_ch", bufs=1, space="PSUM"))
                        ph_cur = [ch_h.tile([P, QCH], F32, tag=f"h{ct}",
                                            name=f"h{ct}")
                                  for ct in range(NCT)]
                        pd_cur = ch_h.tile([P, QCH], F32, tag="d", name="d")
                    # consume p with exp done (2p+1 <= kt), max 2/step
                    target = min((kt - 1) // 2 + 1, NP2)
                    budget = 2
                    while av_done < target and budget > 0:
                        av_step(0, av_done, ph_cur, pd_cur,
                                av_done == 0, av_done == NP2 - 1)
                        av_done += 1
                        budget -= 1
            # finish qc0 chain (p15 needs the last A exp)
            while av_done < NP2:
                av_step(0, av_done, ph_cur, pd_cur,
                        av_done == 0, av_done == NP2 - 1)
                av_done += 1

            # ================= phase B (query half 1) =================
            tail_stage1(0, ph_cur, pd_cur)
            qc_av = 1        # chain currently running
            av_done = 0
            for kt in range(NKT):
                emit_scores_exp(kt, 1)
                if kt == 0:
                    tail_stage2(0)   # frees the qc0 chain PSUM
                if kt == 1:
                    tail_stage3(0, 0)
                if kt == 2:
                    tail_stage3(0, 1)
                # AV for qc1 (burst; all H0..no, all its exps exist) then
                # qc2 (paced behind the B exp sweep)
                if kt >= 1 and qc_av <= 2:
                    if qc_av == 1:
                        target = NP2
                        budget = 3
                    else:
                        target = min((kt - 1) // 2 + 1, NP2)
                        budget = 4
                    while av_done < target and budget > 0:
                        av_step(qc_av, av_done, ph_cur, pd_cur,
                                av_done == 0, av_done == NP2 - 1)
                        av_done += 1
                        budget -= 1
                    if av_done == NP2:
                        tail_stage1(qc_av, ph_cur, pd_cur,
                                    cp_act=(qc_av == 2))
                        tail_stage2(qc_av)
                        if qc_av == 1:
                            qc_av = 2
                            av_done = 0
                        else:
                            qc_av = 3
                if kt == 10:
                    tail_stage3(1, 0)
                if kt == 11:
                    tail_stage3(1, 1)
            # ================= epilogue: qc2 tail + qc3 ===============
            # qc3's denominator accumulates FIRST (the d bank frees as
            # soon as qc2's rec_bc is copied out), so its reciprocal +
            # broadcast chain overlaps the qc3 ph matmuls; epilogue proj
            # matmuls borrow idle scores banks to avoid po-bank churn.
            if qc_av == 2:
                while av_done < NP2:
                    av_step(2, av_done, ph_cur, pd_cur,
                            av_done == 0, av_done == NP2 - 1)
                    av_done += 1
                tail_stage1(2, ph_cur, pd_cur, cp_act=True)
                tail_stage2(2)

            # qc3's accumulators live in the now-idle scores banks so
            # its AV does not wait for qc2's tail to release the chain;
            # its denominator accumulates first so the reciprocal +
            # broadcast chain overlaps the ph matmuls.
            ph3 = ps_s.tile([P, 2 * QCH], F32, tag="s", name="ph3")
            po2 = ps_s.tile([P, 2 * QCH], F32, tag="s", name="po2")
            pd3 = ch_h.tile([P, QCH], F32, tag="po", name="pd3")

            def av3_pd(p, first, last):
                et3 = exp_tiles[p].rearrange(
                    "p (j q) -> p j q", j=2)[:, :, 3 * QCH : 4 * QCH]
                nc.tensor.matmul(
                    pd3[0:1, :], ones3, et3[:],
                    start=first, stop=last, skip_group_check=True,
                    perf_mode=mybir.MatmulPerfMode.DoubleRow)

            def av3_ph(p, first, last):
                et3 = exp_tiles[p].rearrange(
                    "p (j q) -> p j q", j=2)[:, :, 3 * QCH : 4 * QCH]
                vt3 = vT[p].rearrange("p (j c) -> p j c", j=2)
                for ct in range(NCT):
                    nc.tensor.matmul(
                        ph3[:, ct * QCH : (ct + 1) * QCH],
                        vt3[:, :, ct * P : (ct + 1) * P],
                        et3[:], start=first, stop=last,
                        skip_group_check=True,
                        perf_mode=mybir.MatmulPerfMode.DoubleRow)

            for p in range(NP2):
                av3_pd(p, p == 0, p == NP2 - 1)
            tail_stage1(3, None, pd3, cp_act=True)
            for p in range(NP2):
                av3_ph(p, p == 0, p == NP2 - 1)
            tail_stage3(2, 0, po=po2[:, 0:QCH])
            tail_stage3(2, 1, po=po2[:, QCH : 2 * QCH])
            tail_state[3] = ([ph3[:, 0:QCH], ph3[:, QCH : 2 * QCH]],
                             tail_state[3][1])
            tail_stage2(3)
            tail_stage3(3, 0, po=pd_cur[:])
            tail_stage3(3, 1, po=pd3[:])
            p_tail.close()
            p_ch.close()
        p_sw.close()

    _split_excess_waits(nc)
    return nc


def make_in_maps(x, norm_gamma, norm_beta, qkv_w, qkv_b, proj_w, proj_b):
    f = np.float32
    d = np.float64
    qkv_w = np.asarray(qkv_w, dtype=d)
    qkv_b = np.asarray(qkv_b, dtype=d)
    proj_w = np.asarray(proj_w, dtype=d)
    proj_b = np.asarray(proj_b, dtype=d)
    g = np.asarray(norm_gamma, dtype=d)
    beta = np.asarray(norm_beta, dtype=d)
    Wq, Wk, Wv = qkv_w[0:C], qkv_w[C : 2 * C], qkv_w[2 * C : 3 * C]
    bq, bk, bv = qkv_b[0:C], qkv_b[C : 2 * C], qkv_b[2 * C : 3 * C]

    wqT = (Wq.T * g[:, None])          # [c_in, c_out], rows scaled by gamma
    wkT = (Wk.T * g[:, None])
    wvT = (Wv.T * g[:, None])
    u1 = bq + Wq @ beta
    u2 = Wq @ g
    h1 = wkT @ u1
    h2 = wkT @ u2
    dc1 = proj_w @ (bv + Wv @ beta) + proj_b
    pc2 = proj_w @ (Wv @ g)

    wpack = np.zeros((C, PACKW), dtype=f)
    wpack[:, 0:C] = wvT
    wpack[:, C] = h1
    wpack[:, C + 1] = h2
    wpack[:, 258 : 258 + C] = wqT
    wpack[:, 258 + C : 258 + 2 * C] = wkT
    wpack[:, 258 + 2 * C : 258 + 3 * C] = proj_w.T
    wpack[:, 258 + 3 * C] = dc1
    wpack[:, 259 + 3 * C] = pc2
    wpack = np.ascontiguousarray(wpack)

    in_maps = []
    xf = np.asarray(x, dtype=f).reshape(B, C, N)
    for core in range(8):
        b, h = divmod(core, 2)
        xs = xf[b]
        if h == 1:
            xs = np.concatenate([xs[:, NQ:], xs[:, :NQ]], axis=1)
        in_maps.append({"x": np.ascontiguousarray(xs), "wpack": wpack})
    return in_maps


def assemble_output(results):
    out = np.empty((B, C, N), dtype=np.float32)
    for core in range(8):
        b, h = divmod(core, 2)
        out[b][:, h * NQ : (h + 1) * NQ] = results[core]["out"]
    return out.reshape(B, C, HH, WW, DD)


_PROGRAM = None
_N_CALLS = 0
_RUNNER = None


def get_program():
    global _PROGRAM
    if _PROGRAM is None:
        _PROGRAM = build_program()
    return _PROGRAM


def _build_cached_runner(nc):
    """Persistent jitted executor (same execution path that
    run_bass_kernel_spmd takes under axon, via bass2jax/PJRT) so repeat
    kernel() calls skip the multi-minute neuronx-cc recompile."""
    import jax
    from jax.experimental.shard_map import shard_map
    from jax.sharding import Mesh, PartitionSpec
    from concourse import bass2jax

    bass2jax.install_neuronx_cc_hook()
    n_cores = 8
    partition_name = (nc.partition_id_tensor.name
                      if nc.partition_id_tensor else None)
    in_names, out_names, out_avals, zero_outs = [], [], [], []
    for alloc in nc.m.functions[0].allocations:
        if not isinstance(alloc, mybir.MemoryLocationSet):
            continue
        name = alloc.memorylocations[0].name
        if alloc.kind == "ExternalInput":
            if name != partition_name:
                in_names.append(name)
        elif alloc.kind == "ExternalOutput":
            out_names.append(name)
            shape = tuple(alloc.tensor_shape)
            dtype = mybir.dt.np(alloc.dtype)
            out_avals.append(jax.core.ShapedArray(shape, dtype))
            zero_outs.append(np.zeros(shape, dtype))
    n_params = len(in_names)
    all_in_names = list(in_names) + list(out_names)
    if partition_name is not None:
        all_in_names.append(partition_name)

    def _body(*args):
        operands = list(args)
        if partition_name is not None:
            operands.append(bass2jax.partition_id_tensor())
        outs = bass2jax._bass_exec_p.bind(
            *operands,
            out_avals=tuple(out_avals),
            in_names=tuple(all_in_names),
            out_names=tuple(out_names),
            lowering_input_output_aliases=(),
            sim_require_finite=True,
            sim_require_nnan=True,
            nc=nc,
        )
        return tuple(outs)

    devices = jax.devices()[:n_cores]
    mesh = Mesh(np.asarray(devices), ("core",))
    n_outs = len(out_names)
    fn = jax.jit(
        shard_map(_body, mesh=mesh,
                  in_specs=(PartitionSpec("core"),) * (n_params + n_outs),
                  out_specs=(PartitionSpec("core"),) * n_outs,
                  check_rep=False),
        keep_unused=True,
    )

    def run(in_maps):
        per_core = [[np.asarray(m[name]) for name in in_names]
                    for m in in_maps]
        concat_in = [
            np.concatenate([per_core[c][i] for c in range(n_cores)], axis=0)
            for i in range(n_params)
        ]
        concat_zeros = [
            np.zeros((n_cores * z.shape[0], *z.shape[1:]), z.dtype)
            for z in zero_outs
        ]
        out_arrs = fn(*concat_in, *concat_zeros)
        return [
            {name: np.asarray(out_arrs[i]).reshape(
                n_cores, *out_avals[i].shape)[c]
             for i, name in enumerate(out_names)}
            for c in range(n_cores)
        ]

    return run


def kernel(x, norm_gamma, norm_beta, qkv_w, qkv_b, proj_w, proj_b):
    global _N_CALLS, _RUNNER
    nc = get_program()
    in_maps = make_in_maps(x, norm_gamma, norm_beta, qkv_w, qkv_b,
                           proj_w, proj_b)
    _N_CALLS += 1
    if _N_CALLS == 1:
        res = run_bass_kernel_spmd(nc, in_maps, core_ids=list(range(8)))
        return assemble_output(res.results)
    if _RUNNER is None:
        _RUNNER = _build_cached_runner(nc)
    return assemble_output(_RUNNER(in_maps))
